# revision 1
# baseline (speedup 1.0000x reference)
"""Trainium2 Bass kernel for nn_CombinedModel (3-relation GNN with Bernstein
polynomial message passing).

Self-contained: takes full inputs, shards nodes across 8 NeuronCores,
runs a Bass/Tile SPMD program (MLP -> 2 hops of normalized-Laplacian
aggregation -> polynomial projection), gathers the full output.

Structure (v1):
- nodes permuted into (core, window-of-128) slots, degree-balanced
- per relation: h = MLP(x) computed feature-major [128f, NLOC]
- h transposed per window into "wrapped" node-major form nm[128q, NWIN*128f]
  (node j*128+q at partition q, 256B block j) and AllGathered as
  [NCORES*128, NLOC] so every core has the full table
- aggregation: edges grouped by (dst seg of 4 windows, src half), sorted by
  src; per edge a 256B table row is DMA-gathered (flat-row renumbering maps
  (core,window,q) to one contiguous 256B row of the wrapped table); a
  [128e, 512] one-hot (dst offset within seg, scaled by dinv_src*dinv_dst)
  scatters gathered rows into the seg PSUM via TensorE matmul
- W3 applied incrementally (theta-folded) after each hop; relations are
  software-pipelined so AllGathers hide under the previous relation's
  gather phase.
"""
import math
import os
from contextlib import ExitStack

import numpy as np

import concourse.bacc as bacc
import concourse.tile as tile
from concourse import mybir
from concourse.bass_utils import run_bass_kernel_spmd
from concourse.masks import make_identity

F16, F32 = mybir.dt.float16, mybir.dt.float32
I16, I32 = mybir.dt.int16, mybir.dt.int32

NCORES = 8
P = 128
H = 128
IN_FEATS = 256
R = 3
D_ORDER = 2
KORD = D_ORDER + 1
WIN_PER_SEG = 4
MLP_CHUNK = 512
SPLIT = 25600        # flat-row lo/hi split (int16 index range)


def _bernstein_thetas(d):
    thetas = []
    for i in range(d + 1):
        a = np.zeros(i + 1)
        a[i] = 0.5 ** i
        b = np.array([math.comb(d - i, j) * (-0.5) ** j for j in range(d - i + 1)])
        scale = math.factorial(d + 1) / (math.factorial(i) * math.factorial(d - i))
        thetas.append((np.convolve(a, b) * scale).astype(np.float32))
    return np.stack(thetas)  # [d+1, d+1]


THETAS = _bernstein_thetas(D_ORDER)


# ----------------------------------------------------------------------------
# Host-side preprocessing
# ----------------------------------------------------------------------------

def _make_plan(n):
    nloc = ((n + NCORES * P - 1) // (NCORES * P)) * P
    npad = nloc * NCORES
    nwin = nloc // P
    segs = [list(range(s, min(s + WIN_PER_SEG, nwin)))
            for s in range(0, nwin, WIN_PER_SEG)]
    return dict(N=n, NLOC=nloc, NPAD=npad, NWIN=nwin, segs=segs)


def _build_perm(degs_total, npad):
    """Balanced node -> slot permutation. Snake-deal nodes (sorted by total
    degree desc) across all (core, window) bins so per-window edge counts are
    near-equal across cores."""
    n = len(degs_total)
    nloc = npad // NCORES
    nwin_total = npad // P
    tot = np.zeros(npad, np.int64)
    tot[:n] = degs_total
    order = np.argsort(-tot, kind="stable")
    slot_of = np.empty(npad, np.int64)
    counts = np.zeros(nwin_total, np.int64)
    i = np.arange(npad)
    rnd, pos = np.divmod(i, nwin_total)
    w = np.where(rnd % 2 == 0, pos, nwin_total - 1 - pos)
    core = w % NCORES
    j = w // NCORES
    for idx in range(npad):
        g = order[idx]
        ww = w[idx]
        slot_of[g] = core[idx] * nloc + j[idx] * P + counts[ww]
        counts[ww] += 1
    return slot_of


def _pack_idx(idx_flat):
    """[L] int16 -> wrapped [128, L//16] layout (16-partition wrap, replicated)."""
    L = len(idx_flat)
    assert L % 16 == 0
    base = idx_flat.reshape(L // 16, 16).T  # [16, L/16]
    return np.ascontiguousarray(np.tile(base, (8, 1))).astype(np.int16)


def _flat_row(slot, nloc, nwin):
    """node slot (global) -> flat 256B-row index in the wrapped AG table.

    slot = c*nloc + j*128 + q  ->  row (c*128 + q)*nwin + j
    """
    c = slot // nloc
    l = slot % nloc
    j = l // P
    q = l % P
    return (c * P + q) * nwin + j


def _build_streams(plan, s_slot, d_slot, wgt):
    """Per-core gather/one-hot streams for one relation (seg-granular).

    Edges keyed by (dst core, dst seg, src half); sorted by src row within
    each group. Tile stream order: for seg: for part: tiles.
    dq = dst offset within seg (f32), wq = edge weight.
    """
    NLOC, NWIN, segs = plan["NLOC"], plan["NWIN"], plan["segs"]
    nseg = len(segs)
    core = d_slot // NLOC
    lloc = d_slot % NLOC
    seg_of = (lloc // P) // WIN_PER_SEG
    off = (lloc - np.array([s[0] for s in segs])[seg_of] * P).astype(np.float32)
    srow = _flat_row(s_slot, NLOC, NWIN)
    part = (srow >= SPLIT).astype(np.int64)
    key = (core * nseg + seg_of) * 2 + part
    ngroups = NCORES * nseg * 2
    cnt = np.bincount(key, minlength=ngroups).reshape(NCORES, nseg, 2)
    T = np.ceil(cnt / P).astype(np.int64).max(axis=0)  # [nseg, 2]
    order = np.argsort(key, kind="stable")
    gstart = np.zeros(ngroups + 1, np.int64)
    np.cumsum(np.bincount(key, minlength=ngroups), out=gstart[1:])
    ntiles = int(T.sum())

    tile_off = {}
    tpos = 0
    for si in range(nseg):
        for pt in (0, 1):
            tile_off[(si, pt)] = tpos
            tpos += int(T[si, pt])

    idx_vals = srow - part * SPLIT
    order = order[np.lexsort((srow[order], key[order]))]
    per_core = []
    for c in range(NCORES):
        idx_c = np.zeros(ntiles * P, np.int64)
        dq_c = np.full(ntiles * P, -1.0, np.float32)
        wq_c = np.zeros(ntiles * P, np.float32)
        for si in range(nseg):
            for pt in (0, 1):
                g = (c * nseg + si) * 2 + pt
                e = order[gstart[g]:gstart[g + 1]]
                L = len(e)
                if L == 0:
                    continue
                s0 = tile_off[(si, pt)] * P
                idx_c[s0:s0 + L] = idx_vals[e]
                dq_c[s0:s0 + L] = off[e]
                wq_c[s0:s0 + L] = wgt[e]
        per_core.append(dict(
            idx=_pack_idx(idx_c.astype(np.int16)),
            dq=np.ascontiguousarray(dq_c.reshape(ntiles, P).T),
            wq=np.ascontiguousarray(wq_c.reshape(ntiles, P).T),
        ))
    return T, ntiles, per_core


def preprocess(inputs):
    x = np.asarray(inputs["x"], np.float32)
    n = x.shape[0]
    plan = _make_plan(n)
    NLOC, NPAD = plan["NLOC"], plan["NPAD"]

    srcs, dsts, degs = [], [], []
    for r in range(R):
        s = np.asarray(inputs[f"src{r}"]).astype(np.int64)
        d = np.asarray(inputs[f"dst{r}"]).astype(np.int64)
        srcs.append(s)
        dsts.append(d)
        degs.append(np.bincount(d, minlength=n).astype(np.float64))
    perm = _build_perm(sum(degs)[:n].astype(np.int64), NPAD)  # global -> slot

    meta = dict(N=n, NLOC=NLOC, NPAD=NPAD, NWIN=plan["NWIN"],
                segs=tuple(tuple(s) for s in plan["segs"]))
    Ts, ntiles_l, streams = [], [], []
    for r in range(R):
        dinv = 1.0 / np.sqrt(np.maximum(degs[r], 1.0))
        wgt = (dinv[srcs[r]] * dinv[dsts[r]]).astype(np.float32)
        T, ntiles, per_core = _build_streams(
            plan, perm[srcs[r]], perm[dsts[r]], wgt)
        Ts.append(tuple(tuple(int(v) for v in row) for row in T))
        ntiles_l.append(ntiles)
        streams.append(per_core)
    meta["T"] = tuple(Ts)
    meta["ntiles"] = tuple(ntiles_l)

    x_slots = np.zeros((NPAD, IN_FEATS), np.float32)
    x_slots[perm[:n]] = x
    in_maps = []
    weight_names = []
    for r in range(R):
        weight_names += [f"W1_{r}", f"b1_{r}", f"W2_{r}", f"b2_{r}"]
    weight_names += ["W3", "b3"]
    for c in range(NCORES):
        m = {"xT": np.ascontiguousarray(
            x_slots[c * NLOC:(c + 1) * NLOC].T)}
        for name in weight_names:
            m[name] = np.asarray(inputs[name], np.float32)
        for r in range(R):
            m[f"idx{r}"] = streams[r][c]["idx"]
            m[f"dq{r}"] = streams[r][c]["dq"]
            m[f"wq{r}"] = streams[r][c]["wq"]
        in_maps.append(m)
    return meta, in_maps, perm


# ----------------------------------------------------------------------------
# Device program
# ----------------------------------------------------------------------------

def build_program(meta):
    NLOC, NPAD, NWIN = meta["NLOC"], meta["NPAD"], meta["NWIN"]
    segs = [list(s) for s in meta["segs"]]
    nseg = len(segs)
    Ts = [np.array(t, np.int64) for t in meta["T"]]
    ntiles = meta["ntiles"]
    max_ntiles = max(ntiles)

    maxcall = 1
    for r in range(R):
        for si in range(nseg):
            for pt in (0, 1):
                maxcall = max(maxcall, int(Ts[r][si, pt]))

    nc = bacc.Bacc("TRN2", target_bir_lowering=False, debug=False,
                   num_devices=NCORES)

    xT_d = nc.dram_tensor("xT", [IN_FEATS, NLOC], F32, kind="ExternalInput").ap()
    Wd = {}
    for r in range(R):
        Wd[f"W1_{r}"] = nc.dram_tensor(f"W1_{r}", [IN_FEATS, H], F32, kind="ExternalInput").ap()
        Wd[f"b1_{r}"] = nc.dram_tensor(f"b1_{r}", [H], F32, kind="ExternalInput").ap()
        Wd[f"W2_{r}"] = nc.dram_tensor(f"W2_{r}", [H, H], F32, kind="ExternalInput").ap()
        Wd[f"b2_{r}"] = nc.dram_tensor(f"b2_{r}", [H], F32, kind="ExternalInput").ap()
    W3_d = nc.dram_tensor("W3", [KORD * H, H], F32, kind="ExternalInput").ap()
    b3_d = nc.dram_tensor("b3", [H], F32, kind="ExternalInput").ap()
    idx_d, dq_d, wq_d = [], [], []
    for r in range(R):
        idx_d.append(nc.dram_tensor(f"idx{r}", [P, ntiles[r] * 8], I16, kind="ExternalInput").ap())
        dq_d.append(nc.dram_tensor(f"dq{r}", [P, ntiles[r]], F32, kind="ExternalInput").ap())
        wq_d.append(nc.dram_tensor(f"wq{r}", [P, ntiles[r]], F32, kind="ExternalInput").ap())
    out_d = nc.dram_tensor("out", [P, NLOC], F32, kind="ExternalOutput").ap()

    # internal DRAM: AG inputs (wrapped nm form) + shared tables
    aghin, htab, agtin, ttab = [], [], [], []
    for r in range(R):
        aghin.append(nc.dram_tensor(f"aghin{r}", [P, NLOC], F16))
        htab.append(nc.dram_tensor(f"htab{r}", [NCORES * P, NLOC], F16,
                                   addr_space="Shared"))
        agtin.append(nc.dram_tensor(f"agtin{r}", [P, NLOC], F16))
        ttab.append(nc.dram_tensor(f"ttab{r}", [NCORES * P, NLOC], F16,
                                   addr_space="Shared"))

    mlp_chunks = []
    c0 = 0
    while c0 < NLOC:
        cw = min(MLP_CHUNK, NLOC - c0)
        mlp_chunks.append((c0, cw))
        c0 += cw

    with tile.TileContext(nc) as tc, ExitStack() as ctx:
        consts = ctx.enter_context(tc.tile_pool(name="consts", bufs=1))
        wtmp_p = ctx.enter_context(tc.tile_pool(name="wtmp", bufs=2))
        fm_p = ctx.enter_context(tc.tile_pool(name="fm", bufs=1))
        nm_p = ctx.enter_context(tc.tile_pool(name="nm", bufs=1))
        idx_p = ctx.enter_context(tc.tile_pool(name="idxp", bufs=1))
        dq_p = ctx.enter_context(tc.tile_pool(name="dqp", bufs=1))
        v_p = ctx.enter_context(tc.tile_pool(name="vp", bufs=3))
        m_p = ctx.enter_context(tc.tile_pool(name="mp", bufs=5))
        x_p = ctx.enter_context(tc.tile_pool(name="xp", bufs=2))
        h1_p = ctx.enter_context(tc.tile_pool(name="h1p", bufs=2))
        oc_p = ctx.enter_context(tc.tile_pool(name="ocp", bufs=1))
        pp_big = ctx.enter_context(tc.tile_pool(name="ppbig", bufs=3, space="PSUM"))
        pp_hop = ctx.enter_context(tc.tile_pool(name="pphop", bufs=3, space="PSUM"))
        pp_tr = ctx.enter_context(tc.tile_pool(name="pptr", bufs=2, space="PSUM"))

        # ---- constants ----
        iota_i = wtmp_p.tile([P, WIN_PER_SEG * P], I32, tag="iotai")
        nc.gpsimd.iota(iota_i[:], pattern=[[1, WIN_PER_SEG * P]], base=0,
                       channel_multiplier=0)
        iota_f = consts.tile([P, WIN_PER_SEG * P], F16, tag="iotaf")
        nc.vector.tensor_copy(iota_f[:], iota_i[:])
        ident = consts.tile([P, P], F16, tag="ident")
        make_identity(nc, ident[:])

        cast_p = ctx.enter_context(tc.tile_pool(name="castp", bufs=1))

        def load_cast(dst, src_ap, n):
            c0 = 0
            while c0 < n:
                cw = min(512, n - c0)
                tmp = cast_p.tile([P, 512], F32, tag="cast")
                nc.sync.dma_start(out=tmp[:, 0:cw], in_=src_ap[:, c0:c0 + cw])
                nc.any.tensor_copy(dst[:, c0:c0 + cw], tmp[:, 0:cw])
                c0 += cw

        # weights (cast fp16); biases fp32
        W1a, W1b, W2sb, b1c, b2c = [], [], [], [], []
        for r in range(R):
            wa = consts.tile([P, H], F16, tag=f"w1a{r}")
            wb = consts.tile([P, H], F16, tag=f"w1b{r}")
            w2 = consts.tile([P, H], F16, tag=f"w2{r}")
            load_cast(wa, Wd[f"W1_{r}"][0:P, :], H)
            load_cast(wb, Wd[f"W1_{r}"][P:2 * P, :], H)
            load_cast(w2, Wd[f"W2_{r}"][:, :], H)
            b1 = consts.tile([P, 1], F32, tag=f"b1{r}")
            b2 = consts.tile([P, 1], F32, tag=f"b2{r}")
            nc.sync.dma_start(out=b1[:], in_=Wd[f"b1_{r}"][:, None])
            nc.sync.dma_start(out=b2[:], in_=Wd[f"b2_{r}"][:, None])
            W1a.append(wa); W1b.append(wb); W2sb.append(w2)
            b1c.append(b1); b2c.append(b2)

        # W3 folded by Bernstein thetas: W3p_k = sum_j THETA[j,k] * W3_j
        w3s = []
        for jj in range(KORD):
            t = wtmp_p.tile([P, H], F32, tag=f"w3s{jj}")
            nc.sync.dma_start(out=t[:], in_=W3_d[jj * H:(jj + 1) * H, :])
            w3s.append(t)
        W3p = []
        for k in range(KORD):
            acc = wtmp_p.tile([P, H], F32, tag=f"w3acc{k}")
            nc.vector.tensor_scalar(out=acc[:], in0=w3s[0][:],
                                    scalar1=float(THETAS[0, k]), scalar2=None,
                                    op0=mybir.AluOpType.mult)
            for jj in range(1, KORD):
                t2t = wtmp_p.tile([P, H], F32, tag="w3mul")
                nc.vector.tensor_scalar(out=t2t[:], in0=w3s[jj][:],
                                        scalar1=float(THETAS[jj, k]), scalar2=None,
                                        op0=mybir.AluOpType.mult)
                nc.vector.tensor_tensor(out=acc[:], in0=acc[:], in1=t2t[:],
                                        op=mybir.AluOpType.add)
            wk = consts.tile([P, H], F16, tag=f"w3p{k}")
            nc.vector.tensor_copy(wk[:], acc[:])
            W3p.append(wk)
        b3x3 = consts.tile([P, 1], F32, tag="b3x3")
        nc.sync.dma_start(out=b3x3[:], in_=b3_d[:, None])
        nc.vector.tensor_scalar(out=b3x3[:], in0=b3x3[:], scalar1=3.0,
                                scalar2=None, op0=mybir.AluOpType.mult)

        out_acc = consts.tile([P, NLOC], F16, tag="outacc")

        # persistent per-relation feature-major tiles
        hT = []
        T1 = []
        for r in range(R):
            ht_r = consts.tile([P, NLOC], F16, tag=f"ht{r}")
            t1_r = consts.tile([P, NLOC], F16, tag=f"t1{r}")
            hT.append(ht_r)
            T1.append(t1_r)
        T2 = fm_p.tile([P, NLOC], F16, tag="t2")

        # stream tiles (all relations resident; dq/wq in f16 on chip)
        idx_sb = []
        dq_sb = []
        wq_sb = []
        for r in range(R):
            idx_r = idx_p.tile([P, ntiles[r] * 8], I16, tag=f"idx{r}")
            dq_r = dq_p.tile([P, ntiles[r]], F32, tag=f"dq{r}")
            wq_r = dq_p.tile([P, ntiles[r]], F32, tag=f"wq{r}")
            idx_sb.append(idx_r)
            dq_sb.append(dq_r)
            wq_sb.append(wq_r)
        for r in range(R):
            nc.sync.dma_start(out=idx_sb[r][:], in_=idx_d[r][:])
            nc.sync.dma_start(out=dq_sb[r][:], in_=dq_d[r][:])
            nc.sync.dma_start(out=wq_sb[r][:], in_=wq_d[r][:])

        def transpose_to_nm(src_fm):
            nm_tile = nm_p.tile([P, NLOC], F16, tag="nm")
            for j in range(NWIN):
                tp = pp_tr.tile([P, P], F16, space="PSUM", tag="tr")
                nc.tensor.transpose(out=tp[:], in_=src_fm[:, j * P:(j + 1) * P],
                                    identity=ident[:])
                nc.any.tensor_copy(nm_tile[:, j * P:(j + 1) * P], tp[:])
            return nm_tile

        def store_and_allgather(nm_tile, ag_in, table):
            nc.sync.dma_start(out=ag_in.ap(), in_=nm_tile[:])
            if os.environ.get("KNOCC"):
                return
            nc.gpsimd.collective_compute(
                "AllGather", mybir.AluOpType.bypass,
                ins=[ag_in.ap()], outs=[table.ap()],
                replica_groups=[list(range(NCORES))])

        def hop(r, table, prev_fm, next_fm):
            """next_fm = prev_fm - A_hat-weighted gather-scatter of table."""
            T = Ts[r]
            kmode = os.environ.get("KMODE", "full")
            if kmode == "nohop":
                nc.any.tensor_copy(next_fm[:], prev_fm[:])
                return
            # flat 256B-row view of the wrapped table [NCORES*P, NLOC]
            flat = table.ap().rearrange("a (j f) -> (a j) f", f=H)
            lo = flat[0:SPLIT, :]
            hi = flat[SPLIT:NPAD, :]
            icol = 0
            tpos = 0
            for si, seg in enumerate(segs):
                segw = len(seg) * P
                j0 = seg[0]
                tot = int(T[si, 0] + T[si, 1])
                if tot == 0:
                    nc.any.tensor_copy(
                        out=next_fm[:, j0 * P:j0 * P + segw],
                        in_=prev_fm[:, j0 * P:j0 * P + segw])
                    continue
                ps = pp_hop.tile([P, WIN_PER_SEG * P], F32, space="PSUM",
                                 tag="hop")
                k = 0
                for pt, base in ((0, lo), (1, hi)):
                    tcount = int(T[si, pt])
                    if tcount == 0:
                        continue
                    vb = v_p.tile([P, maxcall * P], F16, tag="vbuf")
                    if kmode != "nogather":
                        nc.gpsimd.dma_gather(
                            out_ap=vb[:, 0:tcount * P].rearrange(
                                "p (t e) -> p t e", e=P),
                            in_ap=base,
                            idxs_ap=idx_sb[r][:, icol:icol + tcount * 8],
                            num_idxs=tcount * P,
                            num_idxs_reg=tcount * P,
                            elem_size=H,
                            single_packet=False,
                        )
                    else:
                        nc.vector.memset(vb[:, 0:tcount * P], 0.0)
                    icol += tcount * 8
                    for t in range(tcount):
                        col = tpos + t
                        m = m_p.tile([P, WIN_PER_SEG * P], F16, tag="onehot")
                        nc.any.tensor_scalar(
                            out=m[:, 0:segw], in0=iota_f[:, 0:segw],
                            scalar1=dq_sb[r][:, col:col + 1],
                            scalar2=wq_sb[r][:, col:col + 1],
                            op0=mybir.AluOpType.is_equal,
                            op1=mybir.AluOpType.mult)
                        nc.tensor.matmul(
                            out=ps[:, 0:segw],
                            lhsT=vb[:, t * P:(t + 1) * P],
                            rhs=m[:, 0:segw],
                            start=(k == 0), stop=(k == tot - 1))
                        k += 1
                    tpos += tcount
                nc.any.tensor_tensor(
                    out=next_fm[:, j0 * P:j0 * P + segw],
                    in0=prev_fm[:, j0 * P:j0 * P + segw],
                    in1=ps[:, 0:segw],
                    op=mybir.AluOpType.subtract)

        def w3_accum(src_fm, k, first):
            for (c0, cw) in mlp_chunks:
                psf = pp_big.tile([P, MLP_CHUNK], F32, space="PSUM", tag="big")
                nc.tensor.matmul(out=psf[:, 0:cw], lhsT=W3p[k][:],
                                 rhs=src_fm[:, c0:c0 + cw], start=True, stop=True)
                if first:
                    nc.any.tensor_copy(out_acc[:, c0:c0 + cw], psf[:, 0:cw])
                else:
                    nc.any.tensor_tensor(out=out_acc[:, c0:c0 + cw],
                                         in0=out_acc[:, c0:c0 + cw],
                                         in1=psf[:, 0:cw],
                                         op=mybir.AluOpType.add)

        # ---- phase 1: MLPs for all relations (xT streamed per chunk) ----
        for (c0, cw) in mlp_chunks:
            xa = x_p.tile([P, MLP_CHUNK], F16, tag="xa")
            xb = x_p.tile([P, MLP_CHUNK], F16, tag="xb")
            xtmp = x_p.tile([P, MLP_CHUNK], F32, tag="xtmp")
            nc.sync.dma_start(out=xtmp[:, 0:cw], in_=xT_d[0:P, c0:c0 + cw])
            nc.any.tensor_copy(xa[:, 0:cw], xtmp[:, 0:cw])
            xtmp2 = x_p.tile([P, MLP_CHUNK], F32, tag="xtmp")
            nc.sync.dma_start(out=xtmp2[:, 0:cw], in_=xT_d[P:2 * P, c0:c0 + cw])
            nc.any.tensor_copy(xb[:, 0:cw], xtmp2[:, 0:cw])
            for r in range(R):
                ps1 = pp_big.tile([P, MLP_CHUNK], F32, space="PSUM", tag="big")
                nc.tensor.matmul(out=ps1[:, 0:cw], lhsT=W1a[r][:],
                                 rhs=xa[:, 0:cw], start=True, stop=False)
                nc.tensor.matmul(out=ps1[:, 0:cw], lhsT=W1b[r][:],
                                 rhs=xb[:, 0:cw], start=False, stop=True)
                h1 = h1_p.tile([P, MLP_CHUNK], F16, tag="h1")
                nc.scalar.activation(h1[:, 0:cw], ps1[:, 0:cw],
                                     mybir.ActivationFunctionType.Lrelu,
                                     bias=b1c[r][:], scale=1.0, alpha=0.01)
                ps2 = pp_big.tile([P, MLP_CHUNK], F32, space="PSUM", tag="big")
                nc.tensor.matmul(out=ps2[:, 0:cw], lhsT=W2sb[r][:],
                                 rhs=h1[:, 0:cw], start=True, stop=True)
                nc.scalar.activation(hT[r][:, c0:c0 + cw], ps2[:, 0:cw],
                                     mybir.ActivationFunctionType.Lrelu,
                                     bias=b2c[r][:], scale=1.0, alpha=0.01)

        # ---- phase 2: transpose + AllGather h tables; W3p0 contributions ----
        for r in range(R):
            nm = transpose_to_nm(hT[r])
            store_and_allgather(nm, aghin[r], htab[r])
            w3_accum(hT[r], 0, first=(r == 0))

        # ---- phase 3: hop1 per relation; AG T1; W3p1 contributions ----
        for r in range(R):
            hop(r, htab[r], hT[r], T1[r])
            nm2 = transpose_to_nm(T1[r])
            store_and_allgather(nm2, agtin[r], ttab[r])
            w3_accum(T1[r], 1, first=False)

        # ---- phase 4: hop2 per relation; W3p2 contributions ----
        for r in range(R):
            hop(r, ttab[r], T1[r], T2)
            w3_accum(T2, 2, first=False)

        # ---- output: leaky(out_acc + 3*b3), feat-major ----
        for (c0, cw) in mlp_chunks:
            oc = oc_p.tile([P, MLP_CHUNK], F32, tag="oc")
            nc.scalar.activation(oc[:, 0:cw], out_acc[:, c0:c0 + cw],
                                 mybir.ActivationFunctionType.Lrelu,
                                 bias=b3x3[:], scale=1.0, alpha=0.01)
            nc.sync.dma_start(out=out_d[:, c0:c0 + cw], in_=oc[:, 0:cw])

    nc.compile()
    return nc


# ----------------------------------------------------------------------------
# Entry point
# ----------------------------------------------------------------------------

_prog_cache = {}


def kernel(**inputs):
    meta, in_maps, perm = preprocess(inputs)
    key = repr((meta["N"], meta["NLOC"], meta["T"], meta["ntiles"]))
    if key not in _prog_cache:
        _prog_cache[key] = build_program(meta)
    nc = _prog_cache[key]
    res = run_bass_kernel_spmd(nc, in_maps, list(range(NCORES)))
    outs = [res.results[c]["out"] for c in range(NCORES)]  # [P, NLOC] each
    out_slots = np.concatenate(outs, axis=1).T             # [NPAD, H]
    n = meta["N"]
    return np.ascontiguousarray(out_slots[perm[:n]]).astype(np.float32)

